# revision 7
# baseline (speedup 1.0000x reference)
"""Trainium2 Bass kernel for MQA sliding-window causal self-attention.

Sharding: 8 cores = DP(batch=2) x TP(head-groups=4). Each core computes 4 of
16 query heads for one batch element, shared KV head replicated. Host
pre-packs transposed/padded bf16 layouts; gathers + sums the 4 TP partial
outputs per batch element.

Per-core pipeline (all on-chip data bf16; PSUM accumulation f32):
  phase 1 (8 groups of 2 s-blocks, stage-major software pipeline):
    A: fused QKV+gate projection matmuls (gate folded in as padded 385th col)
    B: Act copies PSUM->SBUF bf16 (qk | v->vext | gate col)
    C: rope muls (DVE tm1, Pool tm2, DVE add)
    D: rmsnorm stats: Act Square, DVE reduce+reciprocal, Act Sqrt
       (k's rstd is NOT applied to k; it is folded into the phase-2 exp scale)
    E: q normalize mul (DVE)
    F: q transposes via DMA xbar-transpose (no PE / no PSUM copies);
       k transpose via PE matmul + Pool copy; kT partition-dup DMA
  boundary: batched gate sigmoid (1 Exp + DVE) + v_eff STT into vext
    -> exactly 2 act-table loads total (Sqrt set, Exp set)
  phase 2 (j = key block; both head-pair passes interleaved per j):
    mm1 scoresT strips (k_j stationary) -> Act exp (scale = k-rstd [128,1] AP)
    -> edge masks (DVE/Pool) -> mm2 accumulating matmuls into 128-partition
    acc: vext cols 64:128 are ones so the softmax denominator lands
    partition-aligned with the output -> DVE reciprocal + elementwise muls
    (no PE broadcast) -> mm3 output projection trails by 2 pairs, y copies
    split DVE/Pool, bf16 DMA out per 2 s-blocks.
"""
import numpy as np
import ml_dtypes
from contextlib import ExitStack

import concourse.bass as bass
import concourse.tile as tile
import concourse.mybir as mybir
from concourse import bacc
from concourse.bass_utils import run_bass_kernel_spmd
from concourse.masks import make_identity

F32 = mybir.dt.float32
BF = mybir.dt.bfloat16
AF = mybir.ActivationFunctionType
ALU = mybir.AluOpType
BF_NP = ml_dtypes.bfloat16

B, S, E, H, KV, D = 2, 2048, 1024, 16, 1, 64
HALF = D // 2
GATE_CH = 32
WIN = 1024
NCORES = 8
TP = 4
HPC = H // TP            # heads per core = 4
HD = HPC * D             # per-core q width = 256
SB = S // 128            # 16 s-blocks
WB = WIN // 128          # 8 window blocks
QKW = HD + 2 * D + 1     # 385 (q 256 | k 64 | v 64 | gate 1)
RW = HD + D              # 320 roped width (4 q heads + k)
GW = 2                   # s-blocks per phase-1 group
NG = SB // GW            # 8 groups

TRACE = False
LAST_RESULT = [None]
_NC_CACHE = [None]


def _build():
    nc = bacc.Bacc()

    xg2 = nc.dram_tensor("xg2", [NG, 128, 8 * 256], BF, kind="ExternalInput")
    wqg = nc.dram_tensor("wqg", [8, 128, QKW], BF, kind="ExternalInput")
    cs2 = nc.dram_tensor("cs2", [128, SB * 128], BF, kind="ExternalInput")
    ve2 = nc.dram_tensor("ve2", [128, SB * 64], BF, kind="ExternalInput")
    wo2 = nc.dram_tensor("wo2", [2, 128, E], BF, kind="ExternalInput")
    mkd = nc.dram_tensor("mkd", [128, 2 * 128], BF, kind="ExternalInput")
    mkf = nc.dram_tensor("mkf", [128, 2 * 128], BF, kind="ExternalInput")
    y = nc.dram_tensor("y", [SB, 128, E], BF, kind="ExternalOutput")

    with tile.TileContext(nc) as tc, ExitStack() as top:
        const = top.enter_context(tc.tile_pool(name="const", bufs=1))
        persist = top.enter_context(tc.tile_pool(name="persist", bufs=1))

        # ---- constants ----
        ident32 = const.tile([128, 128], F32)
        make_identity(nc, ident32)
        ident = const.tile([128, 128], BF)
        nc.vector.tensor_copy(ident, ident32)
        maskd_sb = const.tile([128, 2, 128], BF)
        maskf_sb = const.tile([128, 2, 128], BF)
        nc.sync.dma_start(maskd_sb, mkd[:, :].rearrange("p (h x) -> p h x", h=2))
        nc.sync.dma_start(maskf_sb, mkf[:, :].rearrange("p (h x) -> p h x", h=2))

        wq_all = const.tile([128, 8, QKW], BF)
        for k8 in range(8):
            nc.sync.dma_start(wq_all[:, k8, :], wqg[k8, :, :])
        cs_sb = const.tile([128, SB, 128], BF)
        nc.sync.dma_start(cs_sb, cs2[:, :].rearrange("p (s c) -> p s c", s=SB))
        ve_sb = const.tile([128, SB, 64], BF)
        nc.sync.dma_start(ve_sb, ve2[:, :].rearrange("p (s c) -> p s c", s=SB))
        wo_sb = [const.tile([128, E], BF, name=f"wo{i}") for i in range(2)]
        for i in range(2):
            nc.sync.dma_start(wo_sb[i], wo2[i, :, :])

        # ---- persistent activations ----
        qT = [persist.tile([128, S], BF, name=f"qT{i}") for i in range(2)]
        kT2 = persist.tile([128, S], BF)
        aoT = [persist.tile([128, S], BF, name=f"aoT{i}") for i in range(2)]
        vext = persist.tile([128, SB, 128], BF)
        nc.vector.memset(vext[:, :, 64:128], 1.0)
        rq_sb = persist.tile([128, SB, HPC], F32)
        rk_sb = persist.tile([128, SB], F32)
        gate_sb = persist.tile([128, SB], F32)
        ge = persist.tile([128, SB], F32)
        gd = persist.tile([128, SB], F32)
        sig = persist.tile([128, SB], F32)

        # =============== phase 1: projections + rope + rmsnorm ===============
        with ExitStack() as p1:
            xpool = p1.enter_context(tc.tile_pool(name="xg", bufs=1))
            work = p1.enter_context(tc.tile_pool(name="work", bufs=1))
            qkv_psp = p1.enter_context(tc.tile_pool(name="qkv_ps", bufs=1,
                                                    space="PSUM"))
            ktr_psp = p1.enter_context(tc.tile_pool(name="ktr_ps", bufs=1,
                                                    space="PSUM"))

            xg = {}
            st = {}

            def load_group(g):
                t = xpool.tile([128, 8, 256], BF, tag="xg", name="xg_t", bufs=3)
                nc.sync.dma_start(
                    t, xg2[g, :, :].rearrange("p (k c) -> p k c", k=8))
                xg[g] = t

            def stage_a(g):
                ps_pair = []
                for li in range(GW):
                    lcol = slice(li * 128, (li + 1) * 128)
                    qkv_ps = qkv_psp.tile([128, QKW], F32, tag="qkv",
                                          name="qkv_ps", bufs=3)
                    for k8 in range(8):
                        nc.tensor.matmul(qkv_ps, xg[g][:, k8, lcol],
                                         wq_all[:, k8, :],
                                         start=(k8 == 0), stop=(k8 == 7),
                                         skip_group_check=True)
                    ps_pair.append(qkv_ps)
                st[g] = dict(ps=ps_pair)

            def stage_b(g):
                s_ = st[g]
                qkvsb = work.tile([128, GW, RW], BF, tag="qkvsb",
                                  name="qkvsb", bufs=2)
                for li in range(GW):
                    sb = g * GW + li
                    ps_t = s_["ps"][li]
                    nc.scalar.copy(qkvsb[:, li, :], ps_t[:, 0:RW])
                    nc.scalar.copy(vext[:, sb, 0:64], ps_t[:, RW:RW + 64])
                    nc.scalar.copy(gate_sb[:, sb:sb + 1], ps_t[:, RW + 64:QKW])
                s_["qkvsb"] = qkvsb
                del s_["ps"]

            def csbc(g, off, width):
                # [128, li(2), 5-head bcast, width] view of cos/sin table
                return bass.AP(tensor=cs_sb.tensor,
                               offset=cs_sb.offset + (g * GW) * 128 + off,
                               ap=[list(cs_sb.ap[0]), [128, GW], [0, 5],
                                   [1, width]])

            def stage_c(g):
                s_ = st[g]
                qk5 = s_["qkvsb"][:].rearrange("p l (h d) -> p l h d", h=5)
                tm1 = work.tile([128, GW, RW], BF, tag="tm1", name="tm1",
                                bufs=2)
                tm2 = work.tile([128, GW, RW], BF, tag="tm2", name="tm2",
                                bufs=2)
                tm1v = tm1[:].rearrange("p l (h d) -> p l h d", h=5)
                tm2v = tm2[:].rearrange("p l (h d) -> p l h d", h=5)
                nc.vector.tensor_mul(tm1v, qk5, csbc(g, 0, D))
                nc.gpsimd.tensor_mul(tm2v[:, :, :, 0:HALF],
                                     qk5[:, :, :, HALF:D], csbc(g, D, HALF))
                nc.gpsimd.tensor_mul(tm2v[:, :, :, HALF:D],
                                     qk5[:, :, :, 0:HALF],
                                     csbc(g, D + HALF, HALF))
                qk_r = work.tile([128, GW, RW], BF, tag="qkr", name="qk_r",
                                 bufs=3)
                nc.vector.tensor_add(qk_r, tm1, tm2)
                s_["qk_r"] = qk_r

            def stage_d(g):
                s_ = st[g]
                qk_r = s_["qk_r"]
                sq = work.tile([128, GW, RW], BF, tag="sq", name="sq", bufs=2)
                nc.scalar.square(sq, qk_r)
                ssum = work.tile([128, GW, 5], F32, tag="ssum", name="ssum",
                                 bufs=2)
                nc.vector.reduce_sum(
                    ssum[:].rearrange("p a b -> p (a b)"),
                    sq[:].rearrange("p l (h d) -> p (l h) d", h=5),
                    axis=mybir.AxisListType.X)
                r10 = work.tile([128, GW, 5], F32, tag="r10", name="r10",
                                bufs=2)
                nc.vector.reciprocal(r10, ssum)
                # rstd_q = sqrt(1/ssum)  (D-mean and D^-0.5 fold to scale 1)
                nc.scalar.activation(rq_sb[:, g * GW:(g + 1) * GW, :],
                                     r10[:, :, 0:HPC], AF.Sqrt,
                                     bias=0.0, scale=1.0)
                # rstd_k = sqrt(64/ssum)  (D-mean only; applied in exp scale)
                nc.scalar.activation(
                    rk_sb[:, g * GW:(g + 1) * GW],
                    r10[:, :, HPC:5].rearrange("p a b -> p (a b)"),
                    AF.Sqrt, bias=0.0, scale=float(D))

            def stage_e(g):
                s_ = st[g]
                qn = work.tile([128, GW, HD], BF, tag="qn", name="qn", bufs=3)
                rbc = bass.AP(tensor=rq_sb.tensor,
                              offset=rq_sb.offset + (g * GW) * HPC,
                              ap=[list(rq_sb.ap[0]), [HPC, GW], [1, HPC],
                                  [0, D]])
                nc.vector.tensor_mul(
                    qn[:].rearrange("p l (h d) -> p l h d", h=HPC),
                    s_["qk_r"][:, :, 0:HD].rearrange("p l (h d) -> p l h d",
                                                     h=HPC),
                    rbc)
                s_["qn"] = qn

            def stage_f(g):
                s_ = st.pop(g)
                qn, qk_r = s_["qn"], s_["qk_r"]
                for li in range(GW):
                    sb = g * GW + li
                    scol = slice(sb * 128, (sb + 1) * 128)
                    for i in range(2):
                        nc.sync.dma_start(qT[i][:, scol],
                                          qn[:, li, i * 128:(i + 1) * 128],
                                          transpose=True)
                    ktr = ktr_psp.tile([64, 128], F32, tag="ktr",
                                       name="ktr_ps", bufs=2)
                    nc.tensor.matmul(ktr, qk_r[:, li, HD:RW], ident[:],
                                     start=True, stop=True,
                                     skip_group_check=True)
                    nc.vector.tensor_copy(kT2[0:64, scol], ktr)
                gcol = slice(g * GW * 128, (g + 1) * GW * 128)
                nc.sync.dma_start(kT2[64:128, gcol], kT2[0:64, gcol])

            load_group(0)
            load_group(1)
            for gi in range(NG + 3):
                if gi + 2 < NG:
                    load_group(gi + 2)
                if 0 <= gi - 1 < NG:
                    stage_b(gi - 1)
                if gi < NG:
                    stage_a(gi)
                if 0 <= gi - 1 < NG:
                    stage_c(gi - 1)
                if 0 <= gi - 2 < NG:
                    stage_d(gi - 2)
                    stage_e(gi - 2)
                if 0 <= gi - 3 < NG:
                    stage_f(gi - 3)

            # ---- boundary: gate sigmoid (single Exp) + v_eff ----
            nc.scalar.activation(ge, gate_sb, AF.Exp, bias=0.0, scale=-1.0)
            nc.vector.tensor_scalar_add(gd, ge, 1.0)
            nc.vector.reciprocal(sig, gd)
            for sb in range(SB):
                nc.vector.scalar_tensor_tensor(
                    out=vext[:, sb, 0:64], in0=ve_sb[:, sb, :],
                    scalar=sig[:, sb:sb + 1], in1=vext[:, sb, 0:64],
                    op0=ALU.mult, op1=ALU.add)

        # =============== phase 2: attention + interleaved output proj ========
        with ExitStack() as p2:
            strip_psp = p2.enter_context(tc.tile_pool(name="strip", bufs=1,
                                                      space="PSUM"))
            acc_psp = p2.enter_context(tc.tile_pool(name="acc", bufs=1,
                                                    space="PSUM"))
            y_psp = p2.enter_context(tc.tile_pool(name="y_ps", bufs=1,
                                                  space="PSUM"))
            expp = p2.enter_context(tc.tile_pool(name="expp", bufs=1))
            ep = p2.enter_context(tc.tile_pool(name="ep", bufs=1))
            yp = p2.enter_context(tc.tile_pool(name="yp", bufs=1))

            exps = {0: {}, 1: {}}
            acst = {}

            def mm1(ps, j):
                nq = min(j + WB + 1, SB) - j
                et = expp.tile([128, 2, (WB + 1) * 128], BF, tag=f"exp{ps}",
                               name=f"exp{ps}", bufs=10)
                exps[ps][j] = et
                jcol = slice(j * 128, (j + 1) * 128)
                off = 0
                while off < nq:
                    cn = min(4, nq - off)
                    cw = cn * 128
                    qcol = slice((j + off) * 128, (j + off) * 128 + cw)
                    stp = strip_psp.tile([128, 2, 512], F32, tag="strip",
                                         name="strip", bufs=2)
                    nc.tensor.matmul(stp[:, 0, 0:cw], kT2[0:64, jcol],
                                     qT[ps][0:64, qcol], start=True, stop=True,
                                     skip_group_check=True)
                    nc.tensor.matmul(stp[:, 1, 0:cw], kT2[64:128, jcol],
                                     qT[ps][64:128, qcol], start=True,
                                     stop=True, skip_group_check=True)
                    nc.scalar.activation(et[:, :, off * 128:off * 128 + cw],
                                         stp[:, :, 0:cw], AF.Exp,
                                         bias=0.0, scale=rk_sb[:, j:j + 1])
                    off += cn
                nc.gpsimd.tensor_mul(et[:, :, 0:128], et[:, :, 0:128],
                                     maskd_sb)
                if nq == WB + 1:
                    nc.gpsimd.tensor_mul(et[:, :, WB * 128:(WB + 1) * 128],
                                         et[:, :, WB * 128:(WB + 1) * 128],
                                         maskf_sb)

            def mm2pair(ps, m):
                q0, q1 = 2 * m, 2 * m + 1
                a = acc_psp.tile([128, 2, 256], F32, tag="acc", name="acc",
                                 bufs=2)
                first = True
                if q0 - WB >= 0:
                    jj = q0 - WB
                    nc.tensor.matmul(
                        a[:, :, 0:128], vext[:, jj, :],
                        exps[ps][jj][:, :, (q0 - jj) * 128:(q0 - jj) * 128 + 128],
                        start=True, stop=False, skip_group_check=True)
                    first = False
                for jj in range(max(0, q1 - WB), q0 + 1):
                    off = (q0 - jj) * 128
                    nc.tensor.matmul(a, vext[:, jj, :],
                                     exps[ps][jj][:, :, off:off + 256],
                                     start=first, stop=False,
                                     skip_group_check=True)
                    first = False
                nc.tensor.matmul(a[:, :, 128:256], vext[:, q1, :],
                                 exps[ps][q1][:, :, 0:128],
                                 start=False, stop=True,
                                 skip_group_check=True)
                acst[(ps, m)] = dict(a=a)

            def epi1(ps, m):
                s_ = acst[(ps, m)]
                rec = ep.tile([64, 2, 256], BF, tag=f"rec{ps}",
                              name=f"rec{ps}", bufs=2)
                with nc.allow_low_precision(reason="softmax denom recip"):
                    nc.vector.reciprocal(rec, s_["a"][64:128, :, :])
                s_["rec"] = rec

            def epi2(ps, m):
                s_ = acst.pop((ps, m))
                a, rec = s_["a"], s_["rec"]
                scol = slice(2 * m * 128, (2 * m + 2) * 128)
                aop = ep.tile([64, 2, 256], BF, tag="aop", name="ao_pair",
                              bufs=3)
                nc.vector.tensor_mul(aop, a[0:64, :, :], rec)
                nc.vector.tensor_copy(aoT[ps][0:64, scol], aop[:, 0, :])
                nc.sync.dma_start(aoT[ps][64:128, scol], aop[:, 1, :])

            def mm3(sb, y_t, half):
                scol = slice(sb * 128, (sb + 1) * 128)
                for nch in range(2):
                    y_ps = y_psp.tile([128, 512], F32, tag="y", name="y_ps",
                                      bufs=2)
                    for i in range(2):
                        nc.tensor.matmul(y_ps, aoT[i][:, scol],
                                         wo_sb[i][:, nch * 512:(nch + 1) * 512],
                                         start=(i == 0), stop=(i == 1),
                                         skip_group_check=True)
                    nc.vector.tensor_copy(
                        y_t[:, half, nch * 512:(nch + 1) * 512], y_ps)

            for j in range(SB + 4):
                if j < SB:
                    mm1(0, j)
                    mm1(1, j)
                if j % 2 == 1 and j >= 5 and (j - 5) // 2 < WB:
                    m_ = (j - 5) // 2
                    y_t = yp.tile([128, 2, E], BF, tag="ysb", name="y_t",
                                  bufs=2)
                    mm3(2 * m_, y_t, 0)
                    mm3(2 * m_ + 1, y_t, 1)
                    nc.sync.dma_start(
                        y[2 * m_:2 * m_ + 2, :, :].rearrange("s p e -> p s e"),
                        y_t)
                if j % 2 == 1 and j >= 3 and (j - 3) // 2 < WB:
                    for ps in range(2):
                        epi2(ps, (j - 3) // 2)
                if j % 2 == 1 and (j - 1) // 2 < WB:
                    m = (j - 1) // 2
                    mm2pair(0, m)
                    mm2pair(1, m)
                if j % 2 == 0 and j >= 2 and j // 2 - 1 < WB:
                    for ps in range(2):
                        epi1(ps, j // 2 - 1)

    nc.compile()
    return nc


def _prep_core_inputs(c, x, ve, cos, sin, Wq, Wk, Wv, Wo, Wg):
    b = c // TP
    h0 = (c % TP) * HD
    xT = np.ascontiguousarray(x[b].T).astype(BF_NP)          # [E, S]
    xg2 = np.empty((NG, 128, 8 * 256), BF_NP)
    for g in range(NG):
        for k8 in range(8):
            xg2[g, :, k8 * 256:(k8 + 1) * 256] = \
                xT[k8 * 128:(k8 + 1) * 128, g * 256:(g + 1) * 256]
    wg_pad = np.zeros((E, 1), np.float32)
    wg_pad[:GATE_CH, 0] = Wg[:, 0]
    wqkv = np.concatenate([Wq[:, h0:h0 + HD], Wk, Wv, wg_pad], axis=1)
    wqg = np.ascontiguousarray(wqkv.reshape(8, 128, QKW)).astype(BF_NP)
    ccss = np.concatenate([cos, cos, sin, -sin], axis=1)     # [S, 128]
    cs2 = np.ascontiguousarray(
        ccss.reshape(SB, 128, 128).transpose(1, 0, 2).reshape(128, SB * 128)
    ).astype(BF_NP)
    ve2 = np.ascontiguousarray(
        (2.0 * ve[b]).reshape(SB, 128, 64).transpose(1, 0, 2)
        .reshape(128, SB * 64)).astype(BF_NP)
    wo2 = np.ascontiguousarray(
        Wo[h0:h0 + HD, :].reshape(2, 128, E)).astype(BF_NP)
    ii = np.arange(128)
    md = (ii[None, :] >= ii[:, None]).astype(np.float32)     # [ki, qi]
    mf = 1.0 - md
    mkd = np.tile(md, (1, 2)).reshape(128, 256).astype(BF_NP)
    mkf = np.tile(mf, (1, 2)).reshape(128, 256).astype(BF_NP)
    return dict(xg2=xg2, wqg=wqg, cs2=cs2, ve2=ve2, wo2=wo2,
                mkd=mkd, mkf=mkf)


def kernel(x, ve, cos, sin, Wq, Wk, Wv, Wo, Wg, window_size):
    assert int(window_size) == WIN
    x = np.asarray(x, np.float32)
    ve = np.asarray(ve, np.float32)
    cos = np.asarray(cos, np.float32)
    sin = np.asarray(sin, np.float32)
    Wq = np.asarray(Wq, np.float32)
    Wk = np.asarray(Wk, np.float32)
    Wv = np.asarray(Wv, np.float32)
    Wo = np.asarray(Wo, np.float32)
    Wg = np.asarray(Wg, np.float32)

    if _NC_CACHE[0] is None:
        _NC_CACHE[0] = _build()
    nc = _NC_CACHE[0]

    in_maps = [_prep_core_inputs(c, x, ve, cos, sin, Wq, Wk, Wv, Wo, Wg)
               for c in range(NCORES)]
    res = run_bass_kernel_spmd(nc, in_maps, core_ids=list(range(NCORES)),
                               trace=TRACE)
    LAST_RESULT[0] = res

    out = np.zeros((B, S, E), np.float32)
    for c in range(NCORES):
        out[c // TP] += res.results[c]["y"].astype(np.float32).reshape(S, E)
    return out
